# revision 18
# baseline (speedup 1.0000x reference)
"""Trainium2 Bass kernel for the FFF (fast feedforward / MoE-routing) module.

Math (per token x of dim 1024, PAR=8 trees of 255 nodes):
  logits = x @ W_in.T + b_in                      # [B, 2040]
  dec    = logits > 0
  acts   = silu(logits)
  dmap   = indicator of the 8 visited nodes per tree (root + 7 descents,
           descending by dec at the current node)
  out    = (acts * dmap) @ W_out.T                # [B, 1024]

Strategy (8 NeuronCores, data-parallel over the 8192 tokens, 1024 each):
  - GEMM1 main pass in fp16 (exact products, fp32 PSUM accumulation) over
    all 2040 node columns.  Decision signs for the early tree levels need
    more accuracy than fp16 inputs give (logit err ~1.6e-4 flips ~0.16% of
    near-zero decisions), and a flip at depth d corrupts 7-d downstream
    nodes — so the first 256 node-major columns (levels 0..4) also get a
    bf16 correction pass (eps_x@w + x@eps_w, accumulated into the same
    PSUM group; bf16 residuals need no scaling).  Tile 0 of each core
    skips the correction so the PE is not stalled on the correction-weight
    DMA while the main weights stream in.
  - dmap is built level-by-level with strided vector ops in a node-major
    column layout (col = 8*node + tree): child1 = V_d * dec_d (stride-2
    upsample), child0 = V_d - child1.
  - masked acts cast to fp16, transposed 128x128 on the PE, GEMM2 in fp16
    (exact products, fp32 PSUM accumulation).
  - weight DMAs are chunked and emitted in need-order so the PE starts
    within ~1us instead of waiting for the full weight load.
"""

import numpy as np
import ml_dtypes

DIM = 1024
PAR = 8
DEPTH = 7
N_NODES = 255
WIDTH = PAR * N_NODES          # 2040
NODES_PAD = 2048               # pad masked-acts/W_out^T to 16*128
N_CORES = 8
TOK_PER_CORE = 1024
TT = 128                       # tokens per tile
NTILES = TOK_PER_CORE // TT    # 8
NT_W = 510                     # GEMM1 n-tile width (4 * 510 = 2040)
K_CH = DIM // 128              # 8 contraction chunks for GEMM1
C_CH = NODES_PAD // 128        # 16 contraction chunks for GEMM2
DEC_COLS = 8 * 127             # 1016: decision nodes are levels 0..6
CORR = 256                     # corrected cols (nodes 0..31 = levels 0..4)

_PROGRAM = None


def _build_program():
    import concourse.bacc as bacc
    import concourse.tile as tile
    from concourse import mybir
    from concourse.masks import make_identity
    import concourse.bass as bass

    f32 = mybir.dt.float32
    bf16 = mybir.dt.bfloat16
    f16 = mybir.dt.float16
    Alu = mybir.AluOpType
    Act = mybir.ActivationFunctionType

    nc = bacc.Bacc("TRN2", target_bir_lowering=False, debug=False,
                   num_devices=N_CORES)

    # Per-core DRAM I/O (layouts chosen so every DMA has long contiguous
    # runs).
    xt = nc.dram_tensor("xt", [128, NTILES, K_CH, TT], f16,
                        kind="ExternalInput")
    # bf16 correction operands: [...,0,...] = eps_x, [...,1,...] = x
    xc = nc.dram_tensor("xc", [128, NTILES, 2, K_CH, TT], bf16,
                        kind="ExternalInput")
    w1 = nc.dram_tensor("w1", [128, K_CH, WIDTH], f16, kind="ExternalInput")
    # corr weights: [...,0,:] = bf16(w16[:, :CORR]), [...,1,:] = bf16(eps_w)
    w1c = nc.dram_tensor("w1c", [128, K_CH, 2, CORR], bf16,
                         kind="ExternalInput")
    b1 = nc.dram_tensor("b1", [WIDTH], f32, kind="ExternalInput")
    w2 = nc.dram_tensor("w2", [128, C_CH, DIM], f16, kind="ExternalInput")
    y = nc.dram_tensor("y", [TOK_PER_CORE, DIM], f16, kind="ExternalOutput")

    with tile.TileContext(nc) as tc:
        with (
            tc.tile_pool(name="wts", bufs=1) as wts,
            tc.tile_pool(name="xts", bufs=3) as xts,
            tc.tile_pool(name="logits", bufs=2) as logits_pool,
            tc.tile_pool(name="mask", bufs=2) as mask_pool,
            tc.tile_pool(name="acts", bufs=2) as acts_pool,
            tc.tile_pool(name="out", bufs=2) as out_pool,
            tc.tile_pool(name="pl", bufs=1, space="PSUM") as pl_pool,
            tc.tile_pool(name="pt", bufs=2, space="PSUM") as pt_pool,
            tc.tile_pool(name="py", bufs=2, space="PSUM") as py_pool,
        ):
            # ---- resident weights (DMAs emitted in need-order below) ----
            w1_sb = wts.tile([128, K_CH, WIDTH], f16)
            w1c_sb = wts.tile([128, K_CH, 2, CORR], bf16)
            w2_sb = wts.tile([128, C_CH, DIM], f16)
            b1_sb = wts.tile([128, WIDTH], f32)
            ident = wts.tile([128, 128], f16)

            xt_tiles = {}

            def prefetch_xt(j, eng=None):
                xm = xts.tile([128, K_CH, TT], f16, tag="x")
                xcc = xts.tile([128, 2, K_CH, TT], bf16, tag="xc")
                (eng or nc.sync).dma_start(out=xm, in_=xt[:, j, :, :])
                # only the eps_x half is consumed (the x@eps_w pass reuses
                # the fp16 xm tile as its stationary operand)
                (eng or nc.sync).dma_start(out=xcc[:, 0], in_=xc[:, j, 0, :, :])
                xt_tiles[j] = (xm, xcc)

            # identity first (gpsimd) so the PE warm-up transposes can run
            # while the weight DMAs stream in
            make_identity(nc, ident)

            # Weight DMAs emitted in need-order, split across BOTH DMA
            # dispatchers (each dma_start dispatch costs ~0.65us on its
            # queue; transfers fan out over the 16 shared DMA engines at
            # ~400 GB/s aggregate).  Sync carries the first w1 half + w1c
            # + w2; GpSimd carries the second w1 half + bias + x tiles.
            xm0 = xts.tile([128, K_CH, TT], f16, tag="x")
            nc.sync.dma_start(out=xm0, in_=xt[:, 0, :, :])
            xt_tiles[0] = (xm0, None)
            nc.sync.dma_start(out=w1_sb[:, 0, :], in_=w1[:, 0, :])
            nc.sync.dma_start(out=w1_sb[:, 1, :], in_=w1[:, 1, :])
            for k in range(2, K_CH, 2):
                nc.sync.dma_start(out=w1_sb[:, k:k + 2, :],
                                  in_=w1[:, k:k + 2, :])
            nc.sync.dma_start(out=w1c_sb, in_=w1c[:, :, :, :])
            # bias broadcast rides GpSimd's SW DGE, off the weight path
            b1_bcast = bass.AP(tensor=b1, offset=0, ap=[[0, 128], [1, WIDTH]])
            nc.gpsimd.dma_start(out=b1_sb, in_=b1_bcast)
            prefetch_xt(1, nc.gpsimd)
            for c in range(0, C_CH, 4):
                nc.sync.dma_start(out=w2_sb[:, c:c + 4, :],
                                  in_=w2[:, c:c + 4, :])



            # per-token-tile masked-acts, produced by stage A (GEMM1+mask),
            # consumed by stage B (transpose + GEMM2); 1-deep software
            # pipeline so the PE never waits on the vector-engine epilogue.
            state = {}

            def stage_a(j):
                if j not in xt_tiles:
                    prefetch_xt(j, nc.gpsimd)
                xm, xcc = xt_tiles.pop(j)
                do_corr = j > 0

                lg = logits_pool.tile([TT, WIDTH], f32, tag="lg")
                d1 = mask_pool.tile([TT, DEC_COLS], f16, tag="d1")
                vv = mask_pool.tile([TT, WIDTH], f16, tag="vv")
                ac = acts_pool.tile([TT, WIDTH], f16, tag="ac")
                mk = acts_pool.tile([TT, NODES_PAD], f16, tag="mk")

                # main fp16 pass, k-outer so the PE can start as soon as
                # the first w1 k-chunk lands.  The correction matmuls are
                # emitted right after main k0 (whose full-width start=True
                # clears the PSUM bank) so every group closes at k7 and the
                # epilogue can begin immediately.
                pls = [pl_pool.tile([TT, NT_W], f32, name=f"pl{nt}")
                       for nt in range(4)]
                if j == 0:
                    # ~152 tiny identity matmuls keep the PE continuously
                    # busy (no >3.4us idle window) until the w1 chunks
                    # land, so the HAM clock gate ramps to full speed
                    # during the warm-up and never re-throttles; main
                    # k0's start=True re-clears the bank afterwards
                    for _ in range(152):
                        nc.tensor.matmul(pls[0][:, 0:128], lhsT=ident,
                                         rhs=ident, start=True, stop=True)
                for k in range(K_CH):
                    for nt in range(4):
                        nc.tensor.matmul(
                            pls[nt], lhsT=xm[:, k, :],
                            rhs=w1_sb[:, k, nt * NT_W:(nt + 1) * NT_W],
                            start=(k == 0), stop=(k == K_CH - 1))
                    if k == 0 and do_corr:
                        # bf16 correction into the nt0 PSUM group
                        # (levels 0..4): eps_x@w16 then x16@eps_w (the
                        # second reuses the fp16 xm tile as lhsT)
                        for p in range(2):
                            for kk in range(K_CH):
                                nc.tensor.matmul(
                                    pls[0][:, 0:CORR],
                                    lhsT=(xcc[:, 0, kk, :] if p == 0
                                          else xm[:, kk, :]),
                                    rhs=w1c_sb[:, kk, p, :], start=False,
                                    stop=False)

                for nt in range(4):
                    nsl = slice(nt * NT_W, (nt + 1) * NT_W)
                    # bias add (fp32, exact) PSUM -> SBUF
                    nc.vector.tensor_tensor(lg[:, nsl], pls[nt],
                                            b1_sb[:, nsl], Alu.add)
                    if nt == 0:
                        nc.vector.tensor_scalar(
                            d1[:, 0:NT_W], lg[:, 0:NT_W], 0.0, None,
                            Alu.is_gt)
                    elif nt == 1:
                        nc.vector.tensor_scalar(
                            d1[:, NT_W:DEC_COLS], lg[:, NT_W:DEC_COLS],
                            0.0, None, Alu.is_gt)
                    nc.scalar.activation(ac[:, nsl], lg[:, nsl], Act.Silu)

                # tree mask: V_0 = 1 at root cols; then per level
                # child1 = V_d * dec_d, child0 = V_d - child1
                nc.vector.memset(vv[:, 0:8], 1.0)
                for d in range(DEPTH):
                    ld = 8 * (1 << d)
                    c0 = 8 * ((1 << d) - 1)
                    c1 = 8 * ((1 << (d + 1)) - 1)
                    vpar = vv[:, c0:c0 + ld].rearrange("p (i t) -> p i t", t=8)
                    dpar = d1[:, c0:c0 + ld].rearrange("p (i t) -> p i t", t=8)
                    kids = vv[:, c1:c1 + 2 * ld].rearrange(
                        "p (i two t) -> p i two t", two=2, t=8)
                    nc.vector.tensor_tensor(kids[:, :, 1, :], vpar, dpar,
                                            Alu.mult)
                    nc.vector.tensor_tensor(kids[:, :, 0, :], vpar,
                                            kids[:, :, 1, :], Alu.subtract)

                # masked acts (fp16); cols 2040:2048 are zero padding so the
                # last transpose/GEMM2 chunk is a uniform 128 wide
                nc.vector.memset(mk[:, WIDTH:NODES_PAD], 0.0)
                nc.vector.tensor_tensor(mk[:, 0:1024], ac[:, 0:1024],
                                        vv[:, 0:1024], Alu.mult)
                nc.vector.tensor_tensor(mk[:, 1024:WIDTH], ac[:, 1024:WIDTH],
                                        vv[:, 1024:WIDTH], Alu.mult)
                state[j] = mk

            def stage_b(j):
                mk = state.pop(j)
                at = acts_pool.tile([128, C_CH, TT], f16, tag="at")
                # transpose in groups -> one PSUM tile -> one copy; first
                # group is a single chunk so GEMM2 can start immediately
                c = 0
                for gsz in (1, 3, 4, 4, 4):
                    pt = pt_pool.tile([128, 512], f16)
                    for i in range(gsz):
                        nc.tensor.transpose(
                            pt[:, i * 128:(i + 1) * 128],
                            mk[:, (c + i) * 128:(c + i + 1) * 128], ident)
                    nc.scalar.copy(
                        at[:, c:c + gsz, :],
                        pt[:, :gsz * 128].rearrange("p (c t) -> p c t", t=TT))
                    c += gsz
                ys = out_pool.tile([TT, DIM], f16, tag="ys")
                # the last tile splits the final output half into two
                # 256-wide PSUM groups so the tail cast+DMA is shorter
                if j == NTILES - 1:
                    hslices = [slice(0, 512), slice(512, 768),
                               slice(768, 1024)]
                else:
                    hslices = [slice(0, 512), slice(512, 1024)]
                for hs in hslices:
                    py = py_pool.tile([TT, hs.stop - hs.start], f32,
                                      name="py")
                    for c in range(C_CH):
                        nc.tensor.matmul(
                            py, lhsT=at[:, c, :], rhs=w2_sb[:, c, hs],
                            start=(c == 0), stop=(c == C_CH - 1))
                    nc.vector.tensor_copy(ys[:, hs], py)
                    nc.sync.dma_start(out=y[j * TT:(j + 1) * TT, hs],
                                      in_=ys[:, hs])

            # software pipeline: A(0), A(1), B(0), A(2), B(1), ... B(7)
            stage_a(0)
            for j in range(1, NTILES):
                stage_a(j)
                stage_b(j - 1)
            stage_b(NTILES - 1)

    nc.finalize()
    return nc


def _get_program():
    global _PROGRAM
    if _PROGRAM is None:
        _PROGRAM = _build_program()
    return _PROGRAM


def kernel(oldx, W_in, b_in, W_out):
    from concourse.bass_utils import run_bass_kernel_spmd

    bf16 = ml_dtypes.bfloat16
    oldx = np.asarray(oldx)
    W_in = np.asarray(W_in, dtype=np.float32)
    b_in = np.asarray(b_in, dtype=np.float32)
    W_out = np.asarray(W_out, dtype=np.float32)
    x = oldx.reshape(-1, DIM).astype(np.float32)          # [8192, 1024]

    # node-major column permutation: our col 8n+t  <-  ref col 255t+n
    i = np.arange(WIDTH)
    perm = 255 * (i % PAR) + (i // PAR)

    w1t = W_in[perm, :].T.astype(np.float32)              # [1024, 2040]
    w16 = w1t.astype(np.float16)
    # [dim, width] -> [128, K_CH, WIDTH] with dim = k*128 + p
    w1 = np.ascontiguousarray(
        w16.reshape(K_CH, 128, WIDTH).transpose(1, 0, 2))
    # corr weights for cols 0..CORR: [128, K_CH, 2, CORR]
    wb = w16[:, :CORR].astype(np.float32).astype(bf16)
    ewb = (w1t - w16.astype(np.float32))[:, :CORR].astype(bf16)
    w1c = np.ascontiguousarray(
        np.stack([wb, ewb], axis=1).reshape(K_CH, 128, 2, CORR)
        .transpose(1, 0, 2, 3))
    b1 = np.ascontiguousarray(b_in[perm])

    w2t = np.zeros((NODES_PAD, DIM), np.float32)
    w2t[:WIDTH] = W_out.T[perm, :]
    w2 = np.ascontiguousarray(
        w2t.astype(np.float16).reshape(C_CH, 128, DIM).transpose(1, 0, 2))

    in_maps = []
    for c in range(N_CORES):
        xcf = x[c * TOK_PER_CORE:(c + 1) * TOK_PER_CORE]  # [1024, 1024]
        xT = xcf.T                                        # [dim, tok] f32
        x16 = xT.astype(np.float16)
        ex = (xT - x16.astype(np.float32)).astype(bf16)
        xb = x16.astype(np.float32).astype(bf16)
        # [dim, tok] -> [128, NTILES, K_CH, TT]; dim = k*128+p, tok = j*128+t
        def lay(a):
            return a.reshape(K_CH, 128, NTILES, TT).transpose(1, 2, 0, 3)
        xtc = np.ascontiguousarray(lay(x16))
        xcc = np.ascontiguousarray(
            np.stack([lay(ex), lay(xb)], axis=2))
        in_maps.append({
            "xt": xtc, "xc": xcc, "w1": w1, "w1c": w1c,
            "b1": b1, "w2": w2,
        })

    nc = _get_program()
    res = run_bass_kernel_spmd(nc, in_maps, core_ids=list(range(N_CORES)))
    out = np.concatenate([res.results[c]["y"] for c in range(N_CORES)],
                         axis=0)
    return out.reshape(oldx.shape).astype(np.float32)


# revision 19
# speedup vs baseline: 1.0390x; 1.0390x over previous
"""Trainium2 Bass kernel for the FFF (fast feedforward / MoE-routing) module.

Math (per token x of dim 1024, PAR=8 trees of 255 nodes):
  logits = x @ W_in.T + b_in                      # [B, 2040]
  dec    = logits > 0
  acts   = silu(logits)
  dmap   = indicator of the 8 visited nodes per tree (root + 7 descents,
           descending by dec at the current node)
  out    = (acts * dmap) @ W_out.T                # [B, 1024]

Strategy (8 NeuronCores, data-parallel over the 8192 tokens, 1024 each):
  - GEMM1 main pass in fp16 (exact products, fp32 PSUM accumulation) over
    all 2040 node columns.  Decision signs for the early tree levels need
    more accuracy than fp16 inputs give (logit err ~1.6e-4 flips ~0.16% of
    near-zero decisions), and a flip at depth d corrupts 7-d downstream
    nodes — so the first 256 node-major columns (levels 0..4) also get a
    bf16 correction pass (eps_x@w + x@eps_w, accumulated into the same
    PSUM group; bf16 residuals need no scaling).  Tile 0 of each core
    skips the correction so the PE is not stalled on the correction-weight
    DMA while the main weights stream in.
  - dmap is built level-by-level with strided vector ops in a node-major
    column layout (col = 8*node + tree): child1 = V_d * dec_d (stride-2
    upsample), child0 = V_d - child1.
  - masked acts cast to fp16, transposed 128x128 on the PE, GEMM2 in fp16
    (exact products, fp32 PSUM accumulation).
  - weight DMAs are chunked and emitted in need-order so the PE starts
    within ~1us instead of waiting for the full weight load.
"""

import numpy as np
import ml_dtypes

DIM = 1024
PAR = 8
DEPTH = 7
N_NODES = 255
WIDTH = PAR * N_NODES          # 2040
NODES_PAD = 2048               # pad masked-acts/W_out^T to 16*128
N_CORES = 8
TOK_PER_CORE = 1024
TT = 128                       # tokens per tile
NTILES = TOK_PER_CORE // TT    # 8
NT_W = 510                     # GEMM1 n-tile width (4 * 510 = 2040)
K_CH = DIM // 128              # 8 contraction chunks for GEMM1
C_CH = NODES_PAD // 128        # 16 contraction chunks for GEMM2
DEC_COLS = 8 * 127             # 1016: decision nodes are levels 0..6
CORR = 128                     # corrected cols (nodes 0..15 = levels 0..3)

_PROGRAM = None


def _build_program():
    import concourse.bacc as bacc
    import concourse.tile as tile
    from concourse import mybir
    from concourse.masks import make_identity
    import concourse.bass as bass

    f32 = mybir.dt.float32
    bf16 = mybir.dt.bfloat16
    f16 = mybir.dt.float16
    Alu = mybir.AluOpType
    Act = mybir.ActivationFunctionType

    nc = bacc.Bacc("TRN2", target_bir_lowering=False, debug=False,
                   num_devices=N_CORES)

    # Per-core DRAM I/O (layouts chosen so every DMA has long contiguous
    # runs).
    xt = nc.dram_tensor("xt", [128, NTILES, K_CH, TT], f16,
                        kind="ExternalInput")
    # bf16 correction operands: [...,0,...] = eps_x, [...,1,...] = x
    xc = nc.dram_tensor("xc", [128, NTILES, 2, K_CH, TT], bf16,
                        kind="ExternalInput")
    w1 = nc.dram_tensor("w1", [128, K_CH, WIDTH], f16, kind="ExternalInput")
    # corr weights: [...,0,:] = bf16(w16[:, :CORR]), [...,1,:] = bf16(eps_w)
    w1c = nc.dram_tensor("w1c", [128, K_CH, 2, CORR], bf16,
                         kind="ExternalInput")
    b1 = nc.dram_tensor("b1", [WIDTH], f32, kind="ExternalInput")
    w2 = nc.dram_tensor("w2", [128, C_CH, DIM], f16, kind="ExternalInput")
    y = nc.dram_tensor("y", [TOK_PER_CORE, DIM], f16, kind="ExternalOutput")

    with tile.TileContext(nc) as tc:
        with (
            tc.tile_pool(name="wts", bufs=1) as wts,
            tc.tile_pool(name="xts", bufs=3) as xts,
            tc.tile_pool(name="logits", bufs=2) as logits_pool,
            tc.tile_pool(name="mask", bufs=2) as mask_pool,
            tc.tile_pool(name="acts", bufs=2) as acts_pool,
            tc.tile_pool(name="out", bufs=2) as out_pool,
            tc.tile_pool(name="pl", bufs=1, space="PSUM") as pl_pool,
            tc.tile_pool(name="pt", bufs=2, space="PSUM") as pt_pool,
            tc.tile_pool(name="py", bufs=2, space="PSUM") as py_pool,
        ):
            # ---- resident weights (DMAs emitted in need-order below) ----
            w1_sb = wts.tile([128, K_CH, WIDTH], f16)
            w1c_sb = wts.tile([128, K_CH, 2, CORR], bf16)
            w2_sb = wts.tile([128, C_CH, DIM], f16)
            b1_sb = wts.tile([128, WIDTH], f32)
            ident = wts.tile([128, 128], f16)

            xt_tiles = {}

            def prefetch_xt(j, eng=None):
                xm = xts.tile([128, K_CH, TT], f16, tag="x")
                xcc = xts.tile([128, 2, K_CH, TT], bf16, tag="xc")
                (eng or nc.sync).dma_start(out=xm, in_=xt[:, j, :, :])
                # only the eps_x half is consumed (the x@eps_w pass reuses
                # the fp16 xm tile as its stationary operand)
                (eng or nc.sync).dma_start(out=xcc[:, 0], in_=xc[:, j, 0, :, :])
                xt_tiles[j] = (xm, xcc)

            # identity first (gpsimd) so the PE warm-up transposes can run
            # while the weight DMAs stream in
            make_identity(nc, ident)

            # Weight DMAs emitted in need-order, split across BOTH DMA
            # dispatchers (each dma_start dispatch costs ~0.65us on its
            # queue; transfers fan out over the 16 shared DMA engines at
            # ~400 GB/s aggregate).  Sync carries the first w1 half + w1c
            # + w2; GpSimd carries the second w1 half + bias + x tiles.
            xm0 = xts.tile([128, K_CH, TT], f16, tag="x")
            nc.sync.dma_start(out=xm0, in_=xt[:, 0, :, :])
            xt_tiles[0] = (xm0, None)
            nc.sync.dma_start(out=w1_sb[:, 0, :], in_=w1[:, 0, :])
            nc.sync.dma_start(out=w1_sb[:, 1, :], in_=w1[:, 1, :])
            for k in range(2, K_CH, 2):
                nc.sync.dma_start(out=w1_sb[:, k:k + 2, :],
                                  in_=w1[:, k:k + 2, :])
            nc.sync.dma_start(out=w1c_sb, in_=w1c[:, :, :, :])
            # bias broadcast rides GpSimd's SW DGE, off the weight path
            b1_bcast = bass.AP(tensor=b1, offset=0, ap=[[0, 128], [1, WIDTH]])
            nc.gpsimd.dma_start(out=b1_sb, in_=b1_bcast)
            prefetch_xt(1, nc.gpsimd)
            for c in range(0, C_CH, 4):
                nc.sync.dma_start(out=w2_sb[:, c:c + 4, :],
                                  in_=w2[:, c:c + 4, :])



            # per-token-tile masked-acts, produced by stage A (GEMM1+mask),
            # consumed by stage B (transpose + GEMM2); 1-deep software
            # pipeline so the PE never waits on the vector-engine epilogue.
            state = {}

            def stage_a(j):
                if j not in xt_tiles:
                    prefetch_xt(j, nc.gpsimd)
                xm, xcc = xt_tiles.pop(j)
                do_corr = j > 0

                lg = logits_pool.tile([TT, WIDTH], f32, tag="lg")
                d1 = mask_pool.tile([TT, DEC_COLS], f16, tag="d1")
                vv = mask_pool.tile([TT, WIDTH], f16, tag="vv")
                ac = acts_pool.tile([TT, WIDTH], f16, tag="ac")
                mk = acts_pool.tile([TT, NODES_PAD], f16, tag="mk")

                # main fp16 pass, k-outer so the PE can start as soon as
                # the first w1 k-chunk lands.  The correction matmuls are
                # emitted right after main k0 (whose full-width start=True
                # clears the PSUM bank) so every group closes at k7 and the
                # epilogue can begin immediately.
                pls = [pl_pool.tile([TT, NT_W], f32, name=f"pl{nt}")
                       for nt in range(4)]
                if j == 0:
                    # ~200 tiny identity matmuls keep the PE continuously
                    # busy (no >3.4us idle window) until the w1 chunks
                    # land, so the HAM clock gate ramps to full speed
                    # during the warm-up and never re-throttles; main
                    # k0's start=True re-clears the bank afterwards
                    for _ in range(200):
                        nc.tensor.matmul(pls[0][:, 0:128], lhsT=ident,
                                         rhs=ident, start=True, stop=True)
                for k in range(K_CH):
                    for nt in range(4):
                        nc.tensor.matmul(
                            pls[nt], lhsT=xm[:, k, :],
                            rhs=w1_sb[:, k, nt * NT_W:(nt + 1) * NT_W],
                            start=(k == 0), stop=(k == K_CH - 1))
                    if k == 0 and do_corr:
                        # bf16 correction into the nt0 PSUM group
                        # (levels 0..4): eps_x@w16 then x16@eps_w (the
                        # second reuses the fp16 xm tile as lhsT)
                        for p in range(2):
                            for kk in range(K_CH):
                                nc.tensor.matmul(
                                    pls[0][:, 0:CORR],
                                    lhsT=(xcc[:, 0, kk, :] if p == 0
                                          else xm[:, kk, :]),
                                    rhs=w1c_sb[:, kk, p, :], start=False,
                                    stop=False)

                for nt in range(4):
                    nsl = slice(nt * NT_W, (nt + 1) * NT_W)
                    # bias add (fp32, exact) PSUM -> SBUF
                    nc.vector.tensor_tensor(lg[:, nsl], pls[nt],
                                            b1_sb[:, nsl], Alu.add)
                    if nt == 0:
                        nc.vector.tensor_scalar(
                            d1[:, 0:NT_W], lg[:, 0:NT_W], 0.0, None,
                            Alu.is_gt)
                    elif nt == 1:
                        nc.vector.tensor_scalar(
                            d1[:, NT_W:DEC_COLS], lg[:, NT_W:DEC_COLS],
                            0.0, None, Alu.is_gt)
                    nc.scalar.activation(ac[:, nsl], lg[:, nsl], Act.Silu)

                # tree mask: V_0 = 1 at root cols; then per level
                # child1 = V_d * dec_d, child0 = V_d - child1
                nc.vector.memset(vv[:, 0:8], 1.0)
                for d in range(DEPTH):
                    ld = 8 * (1 << d)
                    c0 = 8 * ((1 << d) - 1)
                    c1 = 8 * ((1 << (d + 1)) - 1)
                    vpar = vv[:, c0:c0 + ld].rearrange("p (i t) -> p i t", t=8)
                    dpar = d1[:, c0:c0 + ld].rearrange("p (i t) -> p i t", t=8)
                    kids = vv[:, c1:c1 + 2 * ld].rearrange(
                        "p (i two t) -> p i two t", two=2, t=8)
                    nc.vector.tensor_tensor(kids[:, :, 1, :], vpar, dpar,
                                            Alu.mult)
                    nc.vector.tensor_tensor(kids[:, :, 0, :], vpar,
                                            kids[:, :, 1, :], Alu.subtract)

                # masked acts (fp16); cols 2040:2048 are zero padding so the
                # last transpose/GEMM2 chunk is a uniform 128 wide
                nc.vector.memset(mk[:, WIDTH:NODES_PAD], 0.0)
                nc.vector.tensor_tensor(mk[:, 0:1024], ac[:, 0:1024],
                                        vv[:, 0:1024], Alu.mult)
                nc.vector.tensor_tensor(mk[:, 1024:WIDTH], ac[:, 1024:WIDTH],
                                        vv[:, 1024:WIDTH], Alu.mult)
                state[j] = mk

            def stage_b(j):
                mk = state.pop(j)
                at = acts_pool.tile([128, C_CH, TT], f16, tag="at")
                # transpose in groups -> one PSUM tile -> one copy; first
                # group is a single chunk so GEMM2 can start immediately
                c = 0
                for gsz in (1, 3, 4, 4, 4):
                    pt = pt_pool.tile([128, 512], f16)
                    for i in range(gsz):
                        nc.tensor.transpose(
                            pt[:, i * 128:(i + 1) * 128],
                            mk[:, (c + i) * 128:(c + i + 1) * 128], ident)
                    nc.scalar.copy(
                        at[:, c:c + gsz, :],
                        pt[:, :gsz * 128].rearrange("p (c t) -> p c t", t=TT))
                    c += gsz
                ys = out_pool.tile([TT, DIM], f16, tag="ys")
                # the last tile splits the final output half into two
                # 256-wide PSUM groups so the tail cast+DMA is shorter
                if j == NTILES - 1:
                    hslices = [slice(0, 512), slice(512, 768),
                               slice(768, 1024)]
                else:
                    hslices = [slice(0, 512), slice(512, 1024)]
                for hs in hslices:
                    py = py_pool.tile([TT, hs.stop - hs.start], f32,
                                      name="py")
                    for c in range(C_CH):
                        nc.tensor.matmul(
                            py, lhsT=at[:, c, :], rhs=w2_sb[:, c, hs],
                            start=(c == 0), stop=(c == C_CH - 1))
                    nc.vector.tensor_copy(ys[:, hs], py)
                    nc.sync.dma_start(out=y[j * TT:(j + 1) * TT, hs],
                                      in_=ys[:, hs])

            # software pipeline: A(0), A(1), B(0), A(2), B(1), ... B(7)
            stage_a(0)
            for j in range(1, NTILES):
                stage_a(j)
                stage_b(j - 1)
            stage_b(NTILES - 1)

    nc.finalize()
    return nc


def _get_program():
    global _PROGRAM
    if _PROGRAM is None:
        _PROGRAM = _build_program()
    return _PROGRAM


def kernel(oldx, W_in, b_in, W_out):
    from concourse.bass_utils import run_bass_kernel_spmd

    bf16 = ml_dtypes.bfloat16
    oldx = np.asarray(oldx)
    W_in = np.asarray(W_in, dtype=np.float32)
    b_in = np.asarray(b_in, dtype=np.float32)
    W_out = np.asarray(W_out, dtype=np.float32)
    x = oldx.reshape(-1, DIM).astype(np.float32)          # [8192, 1024]

    # node-major column permutation: our col 8n+t  <-  ref col 255t+n
    i = np.arange(WIDTH)
    perm = 255 * (i % PAR) + (i // PAR)

    w1t = W_in[perm, :].T.astype(np.float32)              # [1024, 2040]
    w16 = w1t.astype(np.float16)
    # [dim, width] -> [128, K_CH, WIDTH] with dim = k*128 + p
    w1 = np.ascontiguousarray(
        w16.reshape(K_CH, 128, WIDTH).transpose(1, 0, 2))
    # corr weights for cols 0..CORR: [128, K_CH, 2, CORR]
    wb = w16[:, :CORR].astype(np.float32).astype(bf16)
    ewb = (w1t - w16.astype(np.float32))[:, :CORR].astype(bf16)
    w1c = np.ascontiguousarray(
        np.stack([wb, ewb], axis=1).reshape(K_CH, 128, 2, CORR)
        .transpose(1, 0, 2, 3))
    b1 = np.ascontiguousarray(b_in[perm])

    w2t = np.zeros((NODES_PAD, DIM), np.float32)
    w2t[:WIDTH] = W_out.T[perm, :]
    w2 = np.ascontiguousarray(
        w2t.astype(np.float16).reshape(C_CH, 128, DIM).transpose(1, 0, 2))

    in_maps = []
    for c in range(N_CORES):
        xcf = x[c * TOK_PER_CORE:(c + 1) * TOK_PER_CORE]  # [1024, 1024]
        xT = xcf.T                                        # [dim, tok] f32
        x16 = xT.astype(np.float16)
        ex = (xT - x16.astype(np.float32)).astype(bf16)
        xb = x16.astype(np.float32).astype(bf16)
        # [dim, tok] -> [128, NTILES, K_CH, TT]; dim = k*128+p, tok = j*128+t
        def lay(a):
            return a.reshape(K_CH, 128, NTILES, TT).transpose(1, 2, 0, 3)
        xtc = np.ascontiguousarray(lay(x16))
        xcc = np.ascontiguousarray(
            np.stack([lay(ex), lay(xb)], axis=2))
        in_maps.append({
            "xt": xtc, "xc": xcc, "w1": w1, "w1c": w1c,
            "b1": b1, "w2": w2,
        })

    nc = _get_program()
    res = run_bass_kernel_spmd(nc, in_maps, core_ids=list(range(N_CORES)))
    out = np.concatenate([res.results[c]["y"] for c in range(N_CORES)],
                         axis=0)
    return out.reshape(oldx.shape).astype(np.float32)


# revision 21
# speedup vs baseline: 1.0444x; 1.0051x over previous
"""Trainium2 Bass kernel for the FFF (fast feedforward / MoE-routing) module.

Math (per token x of dim 1024, PAR=8 trees of 255 nodes):
  logits = x @ W_in.T + b_in                      # [B, 2040]
  dec    = logits > 0
  acts   = silu(logits)
  dmap   = indicator of the 8 visited nodes per tree (root + 7 descents,
           descending by dec at the current node)
  out    = (acts * dmap) @ W_out.T                # [B, 1024]

Strategy (8 NeuronCores, data-parallel over the 8192 tokens, 1024 each):
  - GEMM1 main pass in fp16 (exact products, fp32 PSUM accumulation) over
    all 2040 node columns.  Decision signs for the early tree levels need
    more accuracy than fp16 inputs give (logit err ~1.6e-4 flips ~0.16% of
    near-zero decisions), and a flip at depth d corrupts 7-d downstream
    nodes — so the first 128 node-major columns (levels 0..3) also get a
    bf16 correction pass (eps_x@w16 + x16@eps_w, accumulated into the
    same PSUM group; bf16 residuals need no scaling, and the x16@eps_w
    matmul reuses the fp16 x tile as its stationary operand).  Tile 0 of
    each core skips the correction so the PE is not stalled on the
    correction-weight DMA while the main weights stream in.
  - dmap is built level-by-level with strided vector ops in a node-major
    column layout (col = 8*node + tree): child1 = V_d * dec_d (stride-2
    upsample), child0 = V_d - child1.
  - masked acts cast to fp16, transposed 128x128 on the PE, GEMM2 in fp16
    (exact products, fp32 PSUM accumulation); output DMA'd as fp16 and
    upcast on the host.
  - weight DMAs are chunked and emitted in need-order on the Sync HW DGE
    (~0.65us dispatch each, transfers fan out over 16 shared DMA engines;
    the head is chip-HBM-bound since all 8 cores pull their weight copies
    at once), and ~200 identity warm-up matmuls keep the PE busy until w1
    lands so the HAM clock gate ramps to 2.4 GHz once and stays there.
"""

import numpy as np
import ml_dtypes

DIM = 1024
PAR = 8
DEPTH = 7
N_NODES = 255
WIDTH = PAR * N_NODES          # 2040
NODES_PAD = 2048               # pad masked-acts/W_out^T to 16*128
N_CORES = 8
TOK_PER_CORE = 1024
TT = 128                       # tokens per tile
NTILES = TOK_PER_CORE // TT    # 8
NT_W = 510                     # GEMM1 n-tile width (4 * 510 = 2040)
K_CH = DIM // 128              # 8 contraction chunks for GEMM1
C_CH = NODES_PAD // 128        # 16 contraction chunks for GEMM2
DEC_COLS = 8 * 127             # 1016: decision nodes are levels 0..6
CORR = 128                     # corrected cols (nodes 0..15 = levels 0..3)

_PROGRAM = None


def _build_program():
    import concourse.bacc as bacc
    import concourse.tile as tile
    from concourse import mybir
    from concourse.masks import make_identity
    import concourse.bass as bass

    f32 = mybir.dt.float32
    bf16 = mybir.dt.bfloat16
    f16 = mybir.dt.float16
    Alu = mybir.AluOpType
    Act = mybir.ActivationFunctionType

    nc = bacc.Bacc("TRN2", target_bir_lowering=False, debug=False,
                   num_devices=N_CORES)

    # Per-core DRAM I/O (layouts chosen so every DMA has long contiguous
    # runs).
    xt = nc.dram_tensor("xt", [128, NTILES, K_CH, TT], f16,
                        kind="ExternalInput")
    # bf16 correction operands: [...,0,...] = eps_x, [...,1,...] = x
    xc = nc.dram_tensor("xc", [128, NTILES, 2, K_CH, TT], bf16,
                        kind="ExternalInput")
    w1 = nc.dram_tensor("w1", [128, K_CH, WIDTH], f16, kind="ExternalInput")
    # corr weights: [...,0,:] = bf16(w16[:, :CORR]), [...,1,:] = bf16(eps_w)
    w1c = nc.dram_tensor("w1c", [128, K_CH, 2, CORR], bf16,
                         kind="ExternalInput")
    b1 = nc.dram_tensor("b1", [WIDTH], f32, kind="ExternalInput")
    w2 = nc.dram_tensor("w2", [128, C_CH, DIM], f16, kind="ExternalInput")
    y = nc.dram_tensor("y", [TOK_PER_CORE, DIM], f16, kind="ExternalOutput")

    with tile.TileContext(nc) as tc:
        with (
            tc.tile_pool(name="wts", bufs=1) as wts,
            tc.tile_pool(name="xts", bufs=3) as xts,
            tc.tile_pool(name="logits", bufs=2) as logits_pool,
            tc.tile_pool(name="mask", bufs=2) as mask_pool,
            tc.tile_pool(name="acts", bufs=2) as acts_pool,
            tc.tile_pool(name="out", bufs=2) as out_pool,
            tc.tile_pool(name="pl", bufs=1, space="PSUM") as pl_pool,
            tc.tile_pool(name="pt", bufs=2, space="PSUM") as pt_pool,
            tc.tile_pool(name="py", bufs=2, space="PSUM") as py_pool,
        ):
            # ---- resident weights (DMAs emitted in need-order below) ----
            w1_sb = wts.tile([128, K_CH, WIDTH], f16)
            w1c_sb = wts.tile([128, K_CH, 2, CORR], bf16)
            w2_sb = wts.tile([128, C_CH, DIM], f16)
            b1_sb = wts.tile([128, WIDTH], f32)
            ident = wts.tile([128, 128], f16)

            xt_tiles = {}

            def prefetch_xt(j, eng=None):
                xm = xts.tile([128, K_CH, TT], f16, tag="x")
                xcc = xts.tile([128, 2, K_CH, TT], bf16, tag="xc")
                (eng or nc.sync).dma_start(out=xm, in_=xt[:, j, :, :])
                # only the eps_x half is consumed (the x@eps_w pass reuses
                # the fp16 xm tile as its stationary operand)
                (eng or nc.sync).dma_start(out=xcc[:, 0], in_=xc[:, j, 0, :, :])
                xt_tiles[j] = (xm, xcc)

            # identity first (gpsimd) so the PE warm-up transposes can run
            # while the weight DMAs stream in
            make_identity(nc, ident)

            # Weight DMAs emitted in need-order, split across BOTH DMA
            # dispatchers (each dma_start dispatch costs ~0.65us on its
            # queue; transfers fan out over the 16 shared DMA engines at
            # ~400 GB/s aggregate).  Sync carries the first w1 half + w1c
            # + w2; GpSimd carries the second w1 half + bias + x tiles.
            nc.sync.dma_start(out=w1_sb[:, 0, :], in_=w1[:, 0, :])
            nc.sync.dma_start(out=w1_sb[:, 1, :], in_=w1[:, 1, :])
            xm0 = xts.tile([128, K_CH, TT], f16, tag="x")
            nc.sync.dma_start(out=xm0, in_=xt[:, 0, :, :])
            xt_tiles[0] = (xm0, None)
            for k in range(2, K_CH, 2):
                nc.sync.dma_start(out=w1_sb[:, k:k + 2, :],
                                  in_=w1[:, k:k + 2, :])
            nc.sync.dma_start(out=w1c_sb, in_=w1c[:, :, :, :])
            # bias broadcast rides GpSimd's SW DGE, off the weight path
            b1_bcast = bass.AP(tensor=b1, offset=0, ap=[[0, 128], [1, WIDTH]])
            nc.gpsimd.dma_start(out=b1_sb, in_=b1_bcast)
            prefetch_xt(1, nc.gpsimd)
            for c in range(0, C_CH, 4):
                nc.sync.dma_start(out=w2_sb[:, c:c + 4, :],
                                  in_=w2[:, c:c + 4, :])



            # per-token-tile masked-acts, produced by stage A (GEMM1+mask),
            # consumed by stage B (transpose + GEMM2); 1-deep software
            # pipeline so the PE never waits on the vector-engine epilogue.
            state = {}

            def stage_a(j):
                if j not in xt_tiles:
                    prefetch_xt(j, nc.gpsimd)
                xm, xcc = xt_tiles.pop(j)
                do_corr = j > 0

                lg = logits_pool.tile([TT, WIDTH], f32, tag="lg")
                d1 = mask_pool.tile([TT, DEC_COLS], f16, tag="d1")
                vv = mask_pool.tile([TT, WIDTH], f16, tag="vv")
                ac = acts_pool.tile([TT, WIDTH], f16, tag="ac")
                mk = acts_pool.tile([TT, NODES_PAD], f16, tag="mk")

                # main fp16 pass, k-outer so the PE can start as soon as
                # the first w1 k-chunk lands.  The correction matmuls are
                # emitted right after main k0 (whose full-width start=True
                # clears the PSUM bank) so every group closes at k7 and the
                # epilogue can begin immediately.
                pls = [pl_pool.tile([TT, NT_W], f32, name=f"pl{nt}")
                       for nt in range(4)]
                if j == 0:
                    # ~200 tiny identity matmuls keep the PE continuously
                    # busy (no >3.4us idle window) until the w1 chunks
                    # land, so the HAM clock gate ramps to full speed
                    # during the warm-up and never re-throttles; main
                    # k0's start=True re-clears the bank afterwards
                    for _ in range(200):
                        nc.tensor.matmul(pls[0][:, 0:128], lhsT=ident,
                                         rhs=ident, start=True, stop=True)
                for k in range(K_CH):
                    for nt in range(4):
                        nc.tensor.matmul(
                            pls[nt], lhsT=xm[:, k, :],
                            rhs=w1_sb[:, k, nt * NT_W:(nt + 1) * NT_W],
                            start=(k == 0), stop=(k == K_CH - 1))
                    if k == 0 and do_corr:
                        # bf16 correction into the nt0 PSUM group
                        # (levels 0..4): eps_x@w16 then x16@eps_w (the
                        # second reuses the fp16 xm tile as lhsT)
                        for p in range(2):
                            for kk in range(K_CH):
                                nc.tensor.matmul(
                                    pls[0][:, 0:CORR],
                                    lhsT=(xcc[:, 0, kk, :] if p == 0
                                          else xm[:, kk, :]),
                                    rhs=w1c_sb[:, kk, p, :], start=False,
                                    stop=False)

                for nt in range(4):
                    nsl = slice(nt * NT_W, (nt + 1) * NT_W)
                    # bias add (fp32, exact) PSUM -> SBUF
                    nc.vector.tensor_tensor(lg[:, nsl], pls[nt],
                                            b1_sb[:, nsl], Alu.add)
                    if nt == 0:
                        nc.vector.tensor_scalar(
                            d1[:, 0:NT_W], lg[:, 0:NT_W], 0.0, None,
                            Alu.is_gt)
                    elif nt == 1:
                        nc.vector.tensor_scalar(
                            d1[:, NT_W:DEC_COLS], lg[:, NT_W:DEC_COLS],
                            0.0, None, Alu.is_gt)
                    nc.scalar.activation(ac[:, nsl], lg[:, nsl], Act.Silu)

                # tree mask: V_0 = 1 at root cols; then per level
                # child1 = V_d * dec_d, child0 = V_d - child1
                nc.vector.memset(vv[:, 0:8], 1.0)
                for d in range(DEPTH):
                    ld = 8 * (1 << d)
                    c0 = 8 * ((1 << d) - 1)
                    c1 = 8 * ((1 << (d + 1)) - 1)
                    vpar = vv[:, c0:c0 + ld].rearrange("p (i t) -> p i t", t=8)
                    dpar = d1[:, c0:c0 + ld].rearrange("p (i t) -> p i t", t=8)
                    kids = vv[:, c1:c1 + 2 * ld].rearrange(
                        "p (i two t) -> p i two t", two=2, t=8)
                    nc.vector.tensor_tensor(kids[:, :, 1, :], vpar, dpar,
                                            Alu.mult)
                    nc.vector.tensor_tensor(kids[:, :, 0, :], vpar,
                                            kids[:, :, 1, :], Alu.subtract)

                # masked acts (fp16); cols 2040:2048 are zero padding so the
                # last transpose/GEMM2 chunk is a uniform 128 wide
                nc.vector.memset(mk[:, WIDTH:NODES_PAD], 0.0)
                nc.vector.tensor_tensor(mk[:, 0:1024], ac[:, 0:1024],
                                        vv[:, 0:1024], Alu.mult)
                nc.vector.tensor_tensor(mk[:, 1024:WIDTH], ac[:, 1024:WIDTH],
                                        vv[:, 1024:WIDTH], Alu.mult)
                state[j] = mk

            def stage_b(j):
                mk = state.pop(j)
                at = acts_pool.tile([128, C_CH, TT], f16, tag="at")
                # transpose in groups -> one PSUM tile -> one copy; first
                # group is a single chunk so GEMM2 can start immediately
                c = 0
                for gsz in (1, 3, 4, 4, 4):
                    pt = pt_pool.tile([128, 512], f16)
                    for i in range(gsz):
                        nc.tensor.transpose(
                            pt[:, i * 128:(i + 1) * 128],
                            mk[:, (c + i) * 128:(c + i + 1) * 128], ident)
                    nc.scalar.copy(
                        at[:, c:c + gsz, :],
                        pt[:, :gsz * 128].rearrange("p (c t) -> p c t", t=TT))
                    c += gsz
                ys = out_pool.tile([TT, DIM], f16, tag="ys")
                # the last tile splits the final output half into two
                # 256-wide PSUM groups so the tail cast+DMA is shorter
                if j == NTILES - 1:
                    hslices = [slice(0, 512), slice(512, 768),
                               slice(768, 1024)]
                else:
                    hslices = [slice(0, 512), slice(512, 1024)]
                for hs in hslices:
                    py = py_pool.tile([TT, hs.stop - hs.start], f32,
                                      name="py")
                    for c in range(C_CH):
                        nc.tensor.matmul(
                            py, lhsT=at[:, c, :], rhs=w2_sb[:, c, hs],
                            start=(c == 0), stop=(c == C_CH - 1))
                    nc.vector.tensor_copy(ys[:, hs], py)
                    nc.sync.dma_start(out=y[j * TT:(j + 1) * TT, hs],
                                      in_=ys[:, hs])

            # software pipeline: A(0), A(1), B(0), A(2), B(1), ... B(7)
            stage_a(0)
            for j in range(1, NTILES):
                stage_a(j)
                stage_b(j - 1)
            stage_b(NTILES - 1)

    nc.finalize()
    return nc


def _get_program():
    global _PROGRAM
    if _PROGRAM is None:
        _PROGRAM = _build_program()
    return _PROGRAM


def kernel(oldx, W_in, b_in, W_out):
    from concourse.bass_utils import run_bass_kernel_spmd

    bf16 = ml_dtypes.bfloat16
    oldx = np.asarray(oldx)
    W_in = np.asarray(W_in, dtype=np.float32)
    b_in = np.asarray(b_in, dtype=np.float32)
    W_out = np.asarray(W_out, dtype=np.float32)
    x = oldx.reshape(-1, DIM).astype(np.float32)          # [8192, 1024]

    # node-major column permutation: our col 8n+t  <-  ref col 255t+n
    i = np.arange(WIDTH)
    perm = 255 * (i % PAR) + (i // PAR)

    w1t = W_in[perm, :].T.astype(np.float32)              # [1024, 2040]
    w16 = w1t.astype(np.float16)
    # [dim, width] -> [128, K_CH, WIDTH] with dim = k*128 + p
    w1 = np.ascontiguousarray(
        w16.reshape(K_CH, 128, WIDTH).transpose(1, 0, 2))
    # corr weights for cols 0..CORR: [128, K_CH, 2, CORR]
    wb = w16[:, :CORR].astype(np.float32).astype(bf16)
    ewb = (w1t - w16.astype(np.float32))[:, :CORR].astype(bf16)
    w1c = np.ascontiguousarray(
        np.stack([wb, ewb], axis=1).reshape(K_CH, 128, 2, CORR)
        .transpose(1, 0, 2, 3))
    b1 = np.ascontiguousarray(b_in[perm])

    w2t = np.zeros((NODES_PAD, DIM), np.float32)
    w2t[:WIDTH] = W_out.T[perm, :]
    w2 = np.ascontiguousarray(
        w2t.astype(np.float16).reshape(C_CH, 128, DIM).transpose(1, 0, 2))

    in_maps = []
    for c in range(N_CORES):
        xcf = x[c * TOK_PER_CORE:(c + 1) * TOK_PER_CORE]  # [1024, 1024]
        xT = xcf.T                                        # [dim, tok] f32
        x16 = xT.astype(np.float16)
        ex = (xT - x16.astype(np.float32)).astype(bf16)
        xb = x16.astype(np.float32).astype(bf16)
        # [dim, tok] -> [128, NTILES, K_CH, TT]; dim = k*128+p, tok = j*128+t
        def lay(a):
            return a.reshape(K_CH, 128, NTILES, TT).transpose(1, 2, 0, 3)
        xtc = np.ascontiguousarray(lay(x16))
        xcc = np.ascontiguousarray(
            np.stack([lay(ex), lay(xb)], axis=2))
        in_maps.append({
            "xt": xtc, "xc": xcc, "w1": w1, "w1c": w1c,
            "b1": b1, "w2": w2,
        })

    nc = _get_program()
    res = run_bass_kernel_spmd(nc, in_maps, core_ids=list(range(N_CORES)))
    out = np.concatenate([res.results[c]["y"] for c in range(N_CORES)],
                         axis=0)
    return out.reshape(oldx.shape).astype(np.float32)


# revision 28
# speedup vs baseline: 1.0980x; 1.0513x over previous
"""Trainium2 Bass kernel for the FFF (fast feedforward / MoE-routing) module.

Math (per token x of dim 1024, PAR=8 trees of 255 nodes):
  logits = x @ W_in.T + b_in                      # [B, 2040]
  dec    = logits > 0
  acts   = silu(logits)
  dmap   = indicator of the 8 visited nodes per tree (root + 7 descents,
           descending by dec at the current node)
  out    = (acts * dmap) @ W_out.T                # [B, 1024]

Strategy (8 NeuronCores, data-parallel over the 8192 tokens, 1024 each):
  - GEMM1 main pass in fp16 (exact products, fp32 PSUM accumulation) over
    all 2040 node columns.  Decision signs for the early tree levels need
    more accuracy than fp16 inputs give (logit err ~1.6e-4 flips ~0.16% of
    near-zero decisions), and a flip at depth d corrupts 7-d downstream
    nodes — so the first 128 node-major columns (levels 0..3) also get a
    bf16 correction pass (eps_x@w16 + x16@eps_w, accumulated into the
    same PSUM group; bf16 residuals need no scaling, and the x16@eps_w
    matmul reuses the fp16 x tile as its stationary operand).  Tile 0 of
    each core skips the correction so the PE is not stalled on the
    correction-weight DMA while the main weights stream in.
  - dmap is built level-by-level with strided vector ops in a node-major
    column layout (col = 8*node + tree): child1 = V_d * dec_d (stride-2
    upsample), child0 = V_d - child1.
  - masked acts cast to fp16, transposed 128x128 on the PE, GEMM2 in fp16
    (exact products, fp32 PSUM accumulation); output DMA'd as fp16 and
    upcast on the host.
  - weight DMAs are chunked and emitted in need-order on the Sync HW DGE
    (~0.65us dispatch each, transfers fan out over 16 shared DMA engines;
    the head is chip-HBM-bound since all 8 cores pull their weight copies
    at once), and ~200 identity warm-up matmuls keep the PE busy until w1
    lands so the HAM clock gate ramps to 2.4 GHz once and stays there.
"""

import numpy as np
import ml_dtypes

DIM = 1024
PAR = 8
DEPTH = 7
N_NODES = 255
WIDTH = PAR * N_NODES          # 2040
NODES_PAD = 2048               # pad masked-acts/W_out^T to 16*128
N_CORES = 8
TOK_PER_CORE = 1024
TT = 128                       # tokens per tile
NTILES = TOK_PER_CORE // TT    # 8
NT_W = 510                     # GEMM1 n-tile width (4 * 510 = 2040)
K_CH = DIM // 128              # 8 contraction chunks for GEMM1
C_CH = NODES_PAD // 128        # 16 contraction chunks for GEMM2
DEC_COLS = 8 * 127             # 1016: decision nodes are levels 0..6
CORR = 128                     # corrected cols (nodes 0..15 = levels 0..3)

_PROGRAM = None


def _build_program():
    import concourse.bacc as bacc
    import concourse.tile as tile
    from concourse import mybir
    from concourse.masks import make_identity
    import concourse.bass as bass

    f32 = mybir.dt.float32
    bf16 = mybir.dt.bfloat16
    f16 = mybir.dt.float16
    Alu = mybir.AluOpType
    Act = mybir.ActivationFunctionType

    nc = bacc.Bacc("TRN2", target_bir_lowering=False, debug=False,
                   num_devices=N_CORES)

    # Per-core DRAM I/O (layouts chosen so every DMA has long contiguous
    # runs).
    xt = nc.dram_tensor("xt", [128, NTILES, K_CH, TT], f16,
                        kind="ExternalInput")
    # bf16 correction operands: [...,0,...] = eps_x, [...,1,...] = x
    xc = nc.dram_tensor("xc", [128, NTILES, 2, K_CH, TT], bf16,
                        kind="ExternalInput")
    w1 = nc.dram_tensor("w1", [128, 4, K_CH, NT_W], f16,
                        kind="ExternalInput")
    # corr weights: [...,0,:] = bf16(w16[:, :CORR]), [...,1,:] = bf16(eps_w)
    w1c = nc.dram_tensor("w1c", [128, K_CH, 2, CORR], bf16,
                         kind="ExternalInput")
    b1 = nc.dram_tensor("b1", [WIDTH], f32, kind="ExternalInput")
    w2 = nc.dram_tensor("w2", [128, C_CH, DIM], f16, kind="ExternalInput")
    y = nc.dram_tensor("y", [TOK_PER_CORE, DIM], f16, kind="ExternalOutput")

    with tile.TileContext(nc) as tc:
        with (
            tc.tile_pool(name="wts", bufs=1) as wts,
            tc.tile_pool(name="xts", bufs=3) as xts,
            tc.tile_pool(name="logits", bufs=2) as logits_pool,
            tc.tile_pool(name="mask", bufs=2) as mask_pool,
            tc.tile_pool(name="acts", bufs=2) as acts_pool,
            tc.tile_pool(name="out", bufs=2) as out_pool,
            tc.tile_pool(name="pl", bufs=1, space="PSUM") as pl_pool,
            tc.tile_pool(name="pt", bufs=2, space="PSUM") as pt_pool,
            tc.tile_pool(name="py", bufs=2, space="PSUM") as py_pool,
        ):
            # ---- resident weights (DMAs emitted in need-order below) ----
            w1_sb = wts.tile([128, 4, K_CH, NT_W], f16)
            w1c_sb = wts.tile([128, K_CH, 2, CORR], bf16)
            w2_sb = wts.tile([128, C_CH, DIM], f16)
            b1_sb = wts.tile([128, WIDTH], f32)
            ident = wts.tile([128, 128], f16)

            xt_tiles = {}

            def prefetch_xt(j, eng=None):
                xm = xts.tile([128, K_CH, TT], f16, tag="x")
                xcc = xts.tile([128, 2, K_CH, TT], bf16, tag="xc")
                (eng or nc.sync).dma_start(out=xm, in_=xt[:, j, :, :])
                # only the eps_x half is consumed (the x@eps_w pass reuses
                # the fp16 xm tile as its stationary operand)
                (eng or nc.sync).dma_start(out=xcc[:, 0], in_=xc[:, j, 0, :, :])
                xt_tiles[j] = (xm, xcc)

            # identity first (gpsimd) so the PE warm-up transposes can run
            # while the weight DMAs stream in
            make_identity(nc, ident)

            # Weight DMAs emitted in need-order, split across BOTH DMA
            # dispatchers (each dma_start dispatch costs ~0.65us on its
            # queue; transfers fan out over the 16 shared DMA engines at
            # ~400 GB/s aggregate).  Sync carries the first w1 half + w1c
            # + w2; GpSimd carries the second w1 half + bias + x tiles.
            # w1 streams in column-blocks (nt-major) so tiles 0+1 can run
            # their GEMM1 nt-by-nt as each 1.05MB block lands, instead of
            # idling until the full weight load completes.
            nc.sync.dma_start(out=w1_sb[:, 0], in_=w1[:, 0])
            xm0 = xts.tile([128, K_CH, TT], f16, tag="x")
            nc.sync.dma_start(out=xm0, in_=xt[:, 0, :, :])
            xt_tiles[0] = (xm0, None)
            xm1 = xts.tile([128, K_CH, TT], f16, tag="x")
            xcc1 = xts.tile([128, 2, K_CH, TT], bf16, tag="xc")
            nc.sync.dma_start(out=xm1, in_=xt[:, 1, :, :])
            nc.sync.dma_start(out=xcc1[:, 0], in_=xc[:, 1, 0, :, :])
            xt_tiles[1] = (xm1, xcc1)
            nc.sync.dma_start(out=w1c_sb, in_=w1c[:, :, :, :])
            for nt in range(1, 4):
                nc.sync.dma_start(out=w1_sb[:, nt], in_=w1[:, nt])
            for c in range(0, C_CH, 4):
                nc.sync.dma_start(out=w2_sb[:, c:c + 4, :],
                                  in_=w2[:, c:c + 4, :])
            # bias broadcast rides GpSimd's SW DGE, off the weight path
            b1_bcast = bass.AP(tensor=b1, offset=0, ap=[[0, 128], [1, WIDTH]])
            nc.gpsimd.dma_start(out=b1_sb, in_=b1_bcast)



            # per-token-tile masked-acts, produced by stage A (GEMM1+mask),
            # consumed by stage B (transpose + GEMM2); 1-deep software
            # pipeline so the PE never waits on the vector-engine epilogue.
            state = {}

            plrr = [0]

            def mask_build(c):
                # tree mask: V_0 = 1 at root cols; then per level
                # child1 = V_d * dec_d, child0 = V_d - child1
                vv, d1 = c["vv"], c["d1"]
                nc.vector.memset(vv[:, 0:8], 1.0)
                for d in range(DEPTH):
                    ld = 8 * (1 << d)
                    c0 = 8 * ((1 << d) - 1)
                    c1 = 8 * ((1 << (d + 1)) - 1)
                    vpar = vv[:, c0:c0 + ld].rearrange("p (i t) -> p i t", t=8)
                    dpar = d1[:, c0:c0 + ld].rearrange("p (i t) -> p i t", t=8)
                    kids = vv[:, c1:c1 + 2 * ld].rearrange(
                        "p (i two t) -> p i two t", two=2, t=8)
                    nc.vector.tensor_tensor(kids[:, :, 1, :], vpar, dpar,
                                            Alu.mult)
                    nc.vector.tensor_tensor(kids[:, :, 0, :], vpar,
                                            kids[:, :, 1, :], Alu.subtract)

            def stage_a(tiles):
                # GEMM1 for a group of token tiles, nt-outer and
                # interleaved across the group so the head tiles can
                # consume each w1 column-block as it lands.  PSUM groups
                # round-robin over the 4 pl names (<=2 open + <=2 in
                # epilogue lag at any time).
                ctx = {}
                for j in tiles:
                    if j not in xt_tiles:
                        prefetch_xt(j, nc.gpsimd)
                    xm, xcc = xt_tiles.pop(j)
                    ctx[j] = {
                        "xm": xm, "xcc": xcc,
                        "lg": logits_pool.tile([TT, WIDTH], f32, tag="lg",
                                               name="lg"),
                        "d1": mask_pool.tile([TT, DEC_COLS], f16, tag="d1",
                                             name="d1"),
                        "vv": mask_pool.tile([TT, WIDTH], f16, tag="vv",
                                             name="vv"),
                        "ac": acts_pool.tile([TT, WIDTH], f16, tag="ac",
                                             name="ac"),
                        "mk": acts_pool.tile([TT, NODES_PAD], f16, tag="mk",
                                             name="mk"),
                    }
                for nt in range(4):
                    nsl = slice(nt * NT_W, (nt + 1) * NT_W)
                    for j in tiles:
                        c = ctx[j]
                        pl = pl_pool.tile([TT, NT_W], f32,
                                          name=f"pl{plrr[0] % 4}")
                        plrr[0] += 1
                        if j == 0 and nt == 0:
                            # ~72 tiny identity matmuls keep the PE busy
                            # and ramp the HAM clock gate while the first
                            # w1 column-block streams in; the start=True
                            # below re-clears the bank
                            for _ in range(72):
                                nc.tensor.matmul(pl[:, 0:128], lhsT=ident,
                                                 rhs=ident, start=True,
                                                 stop=True)
                        for k in range(K_CH):
                            nc.tensor.matmul(
                                pl, lhsT=c["xm"][:, k, :],
                                rhs=w1_sb[:, nt, k, :],
                                start=(k == 0), stop=(k == K_CH - 1))
                            if k == 0 and nt == 0 and j > 0:
                                # bf16 correction into the nt0 PSUM group
                                # (levels 0..3): eps_x@w16 then x16@eps_w
                                # (the second reuses the fp16 xm as lhsT)
                                for p in range(2):
                                    for kk in range(K_CH):
                                        nc.tensor.matmul(
                                            pl[:, 0:CORR],
                                            lhsT=(c["xcc"][:, 0, kk, :]
                                                  if p == 0
                                                  else c["xm"][:, kk, :]),
                                            rhs=w1c_sb[:, kk, p, :],
                                            start=False, stop=False)
                        # bias add (fp32, exact) PSUM -> SBUF
                        nc.vector.tensor_tensor(c["lg"][:, nsl], pl,
                                                b1_sb[:, nsl], Alu.add)
                        if nt == 0:
                            nc.vector.tensor_scalar(
                                c["d1"][:, 0:NT_W], c["lg"][:, 0:NT_W],
                                0.0, None, Alu.is_gt)
                        elif nt == 1:
                            nc.vector.tensor_scalar(
                                c["d1"][:, NT_W:DEC_COLS],
                                c["lg"][:, NT_W:DEC_COLS],
                                0.0, None, Alu.is_gt)
                        nc.scalar.activation(c["ac"][:, nsl],
                                             c["lg"][:, nsl], Act.Silu)
                    if nt == 1:
                        for j in tiles:
                            mask_build(ctx[j])
                # masked acts (fp16); cols 2040:2048 are zero padding so
                # the last transpose/GEMM2 chunk is a uniform 128 wide
                for j in tiles:
                    c = ctx[j]
                    mk, ac, vv = c["mk"], c["ac"], c["vv"]
                    nc.vector.memset(mk[:, WIDTH:NODES_PAD], 0.0)
                    nc.vector.tensor_tensor(mk[:, 0:1024], ac[:, 0:1024],
                                            vv[:, 0:1024], Alu.mult)
                    nc.vector.tensor_tensor(mk[:, 1024:WIDTH],
                                            ac[:, 1024:WIDTH],
                                            vv[:, 1024:WIDTH], Alu.mult)
                    state[j] = mk

            def stage_b(j):
                mk = state.pop(j)
                at = acts_pool.tile([128, C_CH, TT], f16, tag="at")
                # transpose in groups -> one PSUM tile -> one copy; first
                # group is a single chunk so GEMM2 can start immediately
                c = 0
                for gsz in (1, 3, 4, 4, 4):
                    pt = pt_pool.tile([128, 512], f16)
                    for i in range(gsz):
                        nc.tensor.transpose(
                            pt[:, i * 128:(i + 1) * 128],
                            mk[:, (c + i) * 128:(c + i + 1) * 128], ident)
                    nc.scalar.copy(
                        at[:, c:c + gsz, :],
                        pt[:, :gsz * 128].rearrange("p (c t) -> p c t", t=TT))
                    c += gsz
                ys = out_pool.tile([TT, DIM], f16, tag="ys")
                # the last tile splits the final output half into two
                # 256-wide PSUM groups so the tail cast+DMA is shorter
                if j == NTILES - 1:
                    hslices = [slice(0, 512), slice(512, 768),
                               slice(768, 1024)]
                else:
                    hslices = [slice(0, 512), slice(512, 1024)]
                for hs in hslices:
                    py = py_pool.tile([TT, hs.stop - hs.start], f32,
                                      name="py")
                    for c in range(C_CH):
                        nc.tensor.matmul(
                            py, lhsT=at[:, c, :], rhs=w2_sb[:, c, hs],
                            start=(c == 0), stop=(c == C_CH - 1))
                    nc.vector.tensor_copy(ys[:, hs], py)
                    nc.sync.dma_start(out=y[j * TT:(j + 1) * TT, hs],
                                      in_=ys[:, hs])

            # software pipeline: tiles 0+1 interleaved up front (w1
            # streams in), then A(2), B(0), A(3), B(1), ... B(7) —
            # depth 2 at the start so B(0) never waits on the w2 load
            stage_a([0, 1])
            for j in range(2, NTILES):
                stage_a([j])
                stage_b(j - 2)
            stage_b(NTILES - 2)
            stage_b(NTILES - 1)

    nc.finalize()
    return nc


def _get_program():
    global _PROGRAM
    if _PROGRAM is None:
        _PROGRAM = _build_program()
    return _PROGRAM


def kernel(oldx, W_in, b_in, W_out):
    from concourse.bass_utils import run_bass_kernel_spmd

    bf16 = ml_dtypes.bfloat16
    oldx = np.asarray(oldx)
    W_in = np.asarray(W_in, dtype=np.float32)
    b_in = np.asarray(b_in, dtype=np.float32)
    W_out = np.asarray(W_out, dtype=np.float32)
    x = oldx.reshape(-1, DIM).astype(np.float32)          # [8192, 1024]

    # node-major column permutation: our col 8n+t  <-  ref col 255t+n
    i = np.arange(WIDTH)
    perm = 255 * (i % PAR) + (i // PAR)

    w1t = W_in[perm, :].T.astype(np.float32)              # [1024, 2040]
    w16 = w1t.astype(np.float16)
    # [dim, width] -> [128, 4, K_CH, NT_W] (nt-major column blocks)
    w1 = np.ascontiguousarray(
        w16.reshape(K_CH, 128, 4, NT_W).transpose(1, 2, 0, 3))
    # corr weights for cols 0..CORR: [128, K_CH, 2, CORR]
    wb = w16[:, :CORR].astype(np.float32).astype(bf16)
    ewb = (w1t - w16.astype(np.float32))[:, :CORR].astype(bf16)
    w1c = np.ascontiguousarray(
        np.stack([wb, ewb], axis=1).reshape(K_CH, 128, 2, CORR)
        .transpose(1, 0, 2, 3))
    b1 = np.ascontiguousarray(b_in[perm])

    w2t = np.zeros((NODES_PAD, DIM), np.float32)
    w2t[:WIDTH] = W_out.T[perm, :]
    w2 = np.ascontiguousarray(
        w2t.astype(np.float16).reshape(C_CH, 128, DIM).transpose(1, 0, 2))

    in_maps = []
    for c in range(N_CORES):
        xcf = x[c * TOK_PER_CORE:(c + 1) * TOK_PER_CORE]  # [1024, 1024]
        xT = xcf.T                                        # [dim, tok] f32
        x16 = xT.astype(np.float16)
        ex = (xT - x16.astype(np.float32)).astype(bf16)
        xb = x16.astype(np.float32).astype(bf16)
        # [dim, tok] -> [128, NTILES, K_CH, TT]; dim = k*128+p, tok = j*128+t
        def lay(a):
            return a.reshape(K_CH, 128, NTILES, TT).transpose(1, 2, 0, 3)
        xtc = np.ascontiguousarray(lay(x16))
        xcc = np.ascontiguousarray(
            np.stack([lay(ex), lay(xb)], axis=2))
        in_maps.append({
            "xt": xtc, "xc": xcc, "w1": w1, "w1c": w1c,
            "b1": b1, "w2": w2,
        })

    nc = _get_program()
    res = run_bass_kernel_spmd(nc, in_maps, core_ids=list(range(N_CORES)))
    out = np.concatenate([res.results[c]["y"] for c in range(N_CORES)],
                         axis=0)
    return out.reshape(oldx.shape).astype(np.float32)
